# revision 4
# baseline (speedup 1.0000x reference)
"""Trainium2 kernel for BottomUpAttention (gnn_message_passing).

Math note: the reference applies softmax over a singleton axis
(``softmax(scores[:, None], axis=1)``), which is identically 1.0 for every
cell, so the attention branch (cell_keys / tissue_q / tanh / attn_w) cannot
affect the output.  The module reduces exactly to

    out = tissue_features + segment_sum(cell_features, cluster_assignments)

which is a memory-bound scatter-add over 512 MB of cell features.

Strategy (8 NeuronCores, SPMD, no collectives):
  * Shard by *tissue*: each core owns 625 tissues, grouped into 5 blocks of
    125.  Tissues are greedily packed into blocks by descending cell count
    so every block has a near-equal number of cells (minimises padding).
  * Host argsorts cells by tissue id and packs each block's cells into
    128-row tiles, padded to a common tile count T_b so all cores run the
    identical SPMD program.
  * Cell rows are split on the host into bf16 hi + bf16 lo (x = hi + lo,
    max splitting error ~2^-17 relative), interleaved in one array laid out
    partition-major, so the device streams them with fully contiguous
    per-partition DMA descriptors at HBM line rate — same byte count as
    fp32, but the PE runs cheap bf16 matmuls instead of fp32 LOW_HIGH.
  * On device, each 128-cell tile is reduced into its block's [125, 256]
    fp32 PSUM accumulator by two one-hot matmuls (hi and lo):
    lhsT[i, j] = (localid[i] == j), built by one tensor_scalar(is_equal)
    against a constant iota row.  PSUM accumulates in fp32.
  * After a block's tiles are accumulated, PSUM + tissue_features slice is
    written out.  Outputs are [125, 5*256] per core; the host
    inverse-permutes rows into the final [5000, 256].
"""

import numpy as np

P = 128          # SBUF partitions / matmul contraction dim
NCORES = 8
BLK = 125        # tissues per block (PSUM partition rows, <=128)
G = 16           # 128-cell tiles per DMA group (16 -> 2 MiB loads)

LAST_RESULTS = None  # BassKernelResults of the most recent kernel() call

_PROGRAM_CACHE = {}


def _build_program(NT, T_b, NBLK, DIM):
    import concourse.mybir as mybir
    import concourse.tile as tile
    from concourse import bacc

    f32 = mybir.dt.float32
    bf16 = mybir.dt.bfloat16

    nc = bacc.Bacc(
        "TRN2",
        target_bir_lowering=False,
        debug=False,
        enable_asserts=False,
        num_devices=NCORES,
    )
    # hi/lo interleaved cell data, partition-major
    x2 = nc.dram_tensor("x2", [P, NT, 2, DIM], bf16, kind="ExternalInput")
    loc = nc.dram_tensor("loc", [P, NT], f32, kind="ExternalInput")
    iota = nc.dram_tensor("iota", [P, BLK], f32, kind="ExternalInput")
    tqp = nc.dram_tensor("tqp", [BLK, NBLK * DIM], f32, kind="ExternalInput")
    y = nc.dram_tensor("y", [BLK, NBLK * DIM], f32, kind="ExternalOutput")

    with tile.TileContext(nc) as tc:
        with (
            tc.tile_pool(name="const", bufs=1) as cpool,
            tc.tile_pool(name="data", bufs=3) as dpool,
            tc.tile_pool(name="oh", bufs=8) as ohpool,
            tc.tile_pool(name="psum", bufs=2, space="PSUM") as ppool,
        ):
            iota_sb = cpool.tile([P, BLK], f32)
            nc.scalar.dma_start(out=iota_sb[:], in_=iota[:])
            loc_sb = cpool.tile([P, NT], f32)
            nc.scalar.dma_start(out=loc_sb[:], in_=loc[:])
            tqp_sb = cpool.tile([BLK, NBLK * DIM], f32)
            nc.scalar.dma_start(out=tqp_sb[:], in_=tqp[:])
            out_sb = cpool.tile([BLK, NBLK * DIM], f32)

            for b in range(NBLK):
                ps = ppool.tile([BLK, DIM], f32)
                gt0 = b * T_b
                g0 = 0
                while g0 < T_b:
                    gn = min(G, T_b - g0)
                    dt_ = dpool.tile([P, G, 2, DIM], bf16, tag="data")
                    nc.sync.dma_start(
                        out=dt_[:, :gn, :, :],
                        in_=x2[:, gt0 + g0 : gt0 + g0 + gn, :, :],
                    )
                    for t in range(gn):
                        gt = gt0 + g0 + t
                        oh = ohpool.tile([P, BLK], bf16)
                        nc.vector.tensor_scalar(
                            oh[:],
                            iota_sb[:],
                            loc_sb[:, gt : gt + 1],
                            None,
                            mybir.AluOpType.is_equal,
                        )
                        nc.tensor.matmul(
                            out=ps[:],
                            lhsT=oh[:],
                            rhs=dt_[:, t, 0, :],
                            start=(g0 + t == 0),
                            stop=False,
                        )
                        nc.tensor.matmul(
                            out=ps[:],
                            lhsT=oh[:],
                            rhs=dt_[:, t, 1, :],
                            start=False,
                            stop=(g0 + t == T_b - 1),
                        )
                    g0 += gn
                nc.vector.tensor_tensor(
                    out=out_sb[:, b * DIM : (b + 1) * DIM],
                    in0=ps[:],
                    in1=tqp_sb[:, b * DIM : (b + 1) * DIM],
                    op=mybir.AluOpType.add,
                )
            nc.scalar.dma_start(out=y[:], in_=out_sb[:])
    nc.compile()
    return nc


def kernel(
    cell_features,
    tissue_features,
    cluster_assignments,
    W_cell,
    b_cell,
    W_tissue,
    b_tissue,
    attn_w,
):
    global LAST_RESULTS
    import ml_dtypes
    from concourse.bass_utils import run_bass_kernel_spmd

    cells = np.asarray(cell_features, dtype=np.float32)
    tissue = np.asarray(tissue_features, dtype=np.float32)
    assign = np.asarray(cluster_assignments).astype(np.int64)

    n_cell, DIM = cells.shape
    n_tissue = tissue.shape[0]
    assert n_tissue % (NCORES * BLK) == 0, (n_tissue, NCORES, BLK)
    TPC = n_tissue // NCORES       # tissues per core
    NBLK = TPC // BLK              # blocks per core
    nblocks_g = NCORES * NBLK

    # ---- host: bf16 hi/lo split of the cell features ----
    hi = cells.astype(ml_dtypes.bfloat16)
    lo = (cells - hi.astype(np.float32)).astype(ml_dtypes.bfloat16)
    hilo = np.stack([hi, lo], axis=1)          # [n_cell, 2, DIM] bf16

    # ---- host: balance tissues into blocks by cell count (less padding) ----
    tcounts = np.bincount(assign, minlength=n_tissue)
    t_order_desc = np.argsort(-tcounts, kind="stable")
    block_sum = np.zeros(nblocks_g, dtype=np.int64)
    block_fill = np.zeros(nblocks_g, dtype=np.int64)
    tissue2block = np.empty(n_tissue, dtype=np.int64)
    tissue2loc = np.empty(n_tissue, dtype=np.int64)
    import heapq

    heap = [(0, b) for b in range(nblocks_g)]
    heapq.heapify(heap)
    for t in t_order_desc:
        while True:
            s, b = heapq.heappop(heap)
            if block_fill[b] < BLK:
                break
        tissue2block[t] = b
        tissue2loc[t] = block_fill[b]
        block_fill[b] += 1
        block_sum[b] += tcounts[t]
        if block_fill[b] < BLK:
            heapq.heappush(heap, (block_sum[b], b))

    T_b = max(1, int(-(-block_sum.max() // P)))  # tiles per block (all cores)
    CAP = T_b * P
    NT = NBLK * T_b

    # ---- host: sort cells by (block, position) and pack per core ----
    cell_block = tissue2block[assign]
    order = np.argsort(cell_block, kind="stable").astype(np.int64)
    sorted_block = cell_block[order]
    cuts = np.searchsorted(sorted_block, np.arange(nblocks_g + 1))
    loc_of_cell = tissue2loc[assign].astype(np.float32)

    iota_np = np.ascontiguousarray(
        np.tile(np.arange(BLK, dtype=np.float32), (P, 1))
    )
    # tissue rows permuted to (block, localid) layout
    tissue_rows = np.zeros((nblocks_g, BLK, DIM), dtype=np.float32)
    tissue_rows[tissue2block, tissue2loc] = tissue

    in_maps = []
    for k in range(NCORES):
        pi = np.zeros(NBLK * CAP, dtype=np.int64)
        lo_ids = np.full(NBLK * CAP, float(BLK), dtype=np.float32)  # pad -> no hit
        for b in range(NBLK):
            i = k * NBLK + b
            seg = order[cuts[i] : cuts[i + 1]]
            pi[b * CAP : b * CAP + len(seg)] = seg
            lo_ids[b * CAP : b * CAP + len(seg)] = loc_of_cell[seg]
        # partition-major: x2[p, t, :, :] = hilo[pi[t*P + p]]
        x2 = np.ascontiguousarray(hilo[pi.reshape(NT, P).T])
        locT = np.ascontiguousarray(lo_ids.reshape(NT, P).T)
        tqp = np.ascontiguousarray(
            tissue_rows[k * NBLK : (k + 1) * NBLK]
            .transpose(1, 0, 2)
            .reshape(BLK, NBLK * DIM)
        )
        in_maps.append({"x2": x2, "loc": locT, "iota": iota_np, "tqp": tqp})

    # ---- device program (cached on tiling geometry) ----
    key = (NT, T_b, NBLK, DIM)
    nc = _PROGRAM_CACHE.get(key)
    if nc is None:
        nc = _build_program(NT, T_b, NBLK, DIM)
        _PROGRAM_CACHE[key] = nc

    res = run_bass_kernel_spmd(nc, in_maps, core_ids=list(range(NCORES)))
    LAST_RESULTS = res

    # ---- host: inverse-permute per-core outputs into [n_tissue, DIM] ----
    yb = np.concatenate(
        [
            res.results[k]["y"].reshape(BLK, NBLK, DIM).transpose(1, 0, 2)
            for k in range(NCORES)
        ],
        axis=0,
    )  # [nblocks_g, BLK, DIM] in (block, localid) layout
    out = np.ascontiguousarray(yb[tissue2block, tissue2loc])
    return out
